# revision 1
# baseline (speedup 1.0000x reference)
"""Trainium2 Bass kernel for DualGraphConvolution.

reference math (N=8192, D=512):
    node_att = softmax(x @ node_w, axis=0)            # [N, 1]
    h        = x @ edge_w                             # [N, D]
    e        = h @ h.T ; masked where adj <= 0        # [N, N]
    edge_att = softmax(e, axis=1)                     # [N, N]
    out      = (adj * node_att * edge_att) @ (x @ weight) + bias

Distribution: row-shard the N dimension over 8 NeuronCores (1024 rows each).
Each core receives a *rotated* copy of the inputs (its own rows first) so the
SPMD program only ever uses static indices; column order of the j-contraction
is irrelevant because it is summed over.

Per core the kernel computes, for its 1024 rows r:
    m[r, j] = e[r, j] + (adj[r, j] - 1) * 1000        # masked-out cols pushed
                                                      # ~-1000 below the row max
    rowmax, t = exp(m - rowmax), Z = sum t            # online softmax over 4
                                                      # column sweeps of 2048
    O[r, :] = t @ support                             # support = x @ weight
    out = O * (exp(p_r) / (sum_k exp(p_k)) / Z) + bias  # p = x @ node_w

Matmul operands are fp16 (PE runs them at full rate; ~1e-3 relative error),
all accumulation/softmax state is fp32.
"""

import numpy as np

import concourse.bass as bass
import concourse.mybir as mybir
import concourse.tile as tile
from concourse import bacc
from concourse.bass_utils import run_bass_kernel_spmd

F16 = mybir.dt.float16
F32 = mybir.dt.float32
ALU = mybir.AluOpType
ACTF = mybir.ActivationFunctionType
AX = mybir.AxisListType

N = 8192
D = 512
NCORES = 8
JSWEEP = 1024  # columns per online-softmax sweep
NEG_INIT = -3.0e38
MASK_SHIFT = 1000.0  # adj==0 columns get e - 1000: far below row max, exp -> 0


def build_program(n=N, d=D, ncores=NCORES, jsweep=JSWEEP,
                  dbg_skip_main=False, dbg_skip_nodestats=False,
                  dbg_main_upto=None, body_reps=1):
    loc = n // ncores          # rows owned by this core
    rb = loc // 128            # 128-row blocks per core
    kc = d // 128              # contraction chunks of 128
    nsweep = n // jsweep       # online-softmax sweeps
    jt = jsweep // 512         # 512-col j tiles per sweep
    jc = jsweep // 128         # 128-col j chunks per sweep
    rchunks = n // 128
    gw = 8 if jc % 8 == 0 else 4
    assert jc % gw == 0

    nc = bacc.Bacc("TRN2", target_bir_lowering=False, debug=False,
                   num_devices=ncores)

    xt_d = nc.dram_tensor("xt", [kc, 128, n], F16, kind="ExternalInput")
    adj_d = nc.dram_tensor("adj", [loc, n], F16, kind="ExternalInput")
    ew_d = nc.dram_tensor("ew", [kc, 128, d], F16, kind="ExternalInput")
    wt_d = nc.dram_tensor("wt", [kc, 128, d], F16, kind="ExternalInput")
    nw_d = nc.dram_tensor("nw", [kc, 128, 1], F16, kind="ExternalInput")
    bias_d = nc.dram_tensor("biasb", [128, d], F32, kind="ExternalInput")
    id_d = nc.dram_tensor("ident", [128, 128], F16, kind="ExternalInput")
    idk_d = nc.dram_tensor("identk", [128, 128], F16, kind="ExternalInput")
    out_d = nc.dram_tensor("out", [loc, d], F32, kind="ExternalOutput")

    with tile.TileContext(nc) as tc:
        for _rep in range(body_reps):
            with (
                tc.tile_pool(name="const", bufs=1) as constp,
                tc.tile_pool(name="big", bufs=1) as bigp,
                tc.tile_pool(name="vec", bufs=6) as vecp,
            ):
                ew_sb = constp.tile([128, kc, d], F16)
                nc.sync.dma_start(ew_sb[:], ew_d.rearrange("c p f -> p c f"))
                wt_sb = constp.tile([128, kc, d], F16)
                nc.sync.dma_start(wt_sb[:], wt_d.rearrange("c p f -> p c f"))
                nw_sb = constp.tile([128, kc, 1], F16)
                nc.sync.dma_start(nw_sb[:], nw_d.rearrange("c p f -> p c f"))
                bias_sb = constp.tile([128, d], F32)
                nc.sync.dma_start(bias_sb[:], bias_d[:])
                id_sb = constp.tile([128, 128], F16)
                nc.sync.dma_start(id_sb[:], id_d[:])
                idk_sb = constp.tile([128, 128], F16)
                nc.sync.dma_start(idk_sb[:], idk_d[:])

                hT_sb = bigp.tile([128, kc, n], F16)   # h[r, dd] at [dd%128, dd//128, r]
                sup_sb = bigp.tile([128, rchunks, d], F16)  # support[rc*128+p, f]
                pzp = bigp.tile([1, n // 512], F32)    # per-r-tile sums of exp(p)
                ploc_sb = bigp.tile([128, rb], F32)    # exp(p) for local rows
                ones_row = constp.tile([1, 128], F32)
                nc.vector.memset(ones_row[:], 1.0)

                # ---- phase 0 (replicated): hT, support, p for all rows ----
                xt_view = xt_d.rearrange("c p r -> p c r")
                with (
                    tc.tile_pool(name="ph0", bufs=3) as ph0p,
                    tc.tile_pool(name="ph0ps", bufs=2, space="PSUM") as ph0ps,
                    tc.tile_pool(name="ph0ps1", bufs=1, space="PSUM") as ph0ps1,
                ):
                    for rt in range(n // 512):
                        xt_t = ph0p.tile([128, kc, 512], F16, tag="xt")
                        nc.sync.dma_start(
                            xt_t[:], xt_view[:, :, rt * 512:(rt + 1) * 512])
                        for dcp in range(kc // 2):
                            # two 512-wide groups into one 2-bank psum tile,
                            # one strided copy writes both hT d-chunks
                            hps = ph0ps.tile([128, 1024], F32, tag="hps")
                            for half in range(2):
                                dc = dcp * 2 + half
                                for c in range(kc):
                                    nc.tensor.matmul(
                                        hps[:, half * 512:(half + 1) * 512],
                                        ew_sb[:, c, dc * 128:(dc + 1) * 128],
                                        xt_t[:, c, :], start=(c == 0),
                                        stop=(c == kc - 1))
                            nc.vector.tensor_copy(
                                out=hT_sb[:, dcp * 2:dcp * 2 + 2,
                                          rt * 512:(rt + 1) * 512],
                                in_=hps[:].rearrange("p (h f) -> p h f", h=2))
                        for rs in range(4):
                            rch = rt * 4 + rs
                            sps = ph0ps.tile([128, d], F32, tag="sps")
                            for c in range(kc):
                                nc.tensor.matmul(
                                    sps[:], xt_t[:, c, rs * 128:(rs + 1) * 128],
                                    wt_sb[:, c, :], start=(c == 0), stop=(c == kc - 1))
                            nc.vector.tensor_copy(out=sup_sb[:, rch, :],
                                                  in_=sps[:])
                        # p slice [1, 512] via nw as the 1-col stationary
                        pps = ph0ps1.tile([1, 512], F32, tag="pps")
                        for c in range(kc):
                            nc.tensor.matmul(
                                pps[:], nw_sb[:, c, :], xt_t[:, c, :],
                                start=(c == 0), stop=(c == kc - 1))
                        pe_t = ph0p.tile([1, 512], F32, tag="pe")
                        nc.scalar.activation(pe_t[:], pps[:], ACTF.Exp,
                                             accum_out=pzp[:, rt:rt + 1])
                        if rt * 512 < loc:
                            # local rows: spread exp(p) across partitions via
                            # K=1 matmuls with the [1,128] slice stationary
                            for i in range(4):
                                b0 = rt * 4 + i
                                if b0 >= rb:
                                    break
                                tp_ps = ph0ps1.tile([128, 1], F32, tag="ptp")
                                nc.tensor.matmul(
                                    tp_ps[:], pe_t[:, i * 128:(i + 1) * 128],
                                    ones_row[:, 0:1])
                                nc.vector.tensor_copy(
                                    out=ploc_sb[:, b0:b0 + 1], in_=tp_ps[:])

                # ---- node attention: scale0 = exp(p_loc) / sum(exp(p)) ----
                scale0 = bigp.tile([128, rb], F32)
                if dbg_skip_nodestats:
                    nc.vector.memset(scale0[:], 1.0)
                else:
                    pz = vecp.tile([1, 1], F32, tag="pz")
                    nc.vector.reduce_sum(pz[:], pzp[:], axis=AX.X)
                    pzi = vecp.tile([1, 1], F32, tag="pzi")
                    nc.vector.reciprocal(pzi[:], pz[:])
                    with tc.tile_pool(name="nps", bufs=2, space="PSUM") as npsp:
                        # broadcast 1/pz to all partitions via K=1 matmul
                        pzb_ps = npsp.tile([128, 1], F32, tag="pzbps")
                        nc.tensor.matmul(pzb_ps[:], ones_row[:], pzi[:])
                        pzb = vecp.tile([128, 1], F32, tag="pzb")
                        nc.vector.tensor_copy(out=pzb[:], in_=pzb_ps[:])
                    nc.vector.tensor_scalar_mul(scale0[:], ploc_sb[:], pzb[:])

                if dbg_skip_main:
                    with tc.tile_pool(name="dbgo", bufs=2) as dbgo:
                        for b in range(rb):
                            o_t = dbgo.tile([128, d], F32, tag="o")
                            nc.vector.tensor_scalar_mul(o_t[:], sup_sb[:, b, :],
                                                        scale0[:, b:b + 1])
                            nc.sync.dma_start(out_d[b * 128:(b + 1) * 128, :],
                                              o_t[:])

                # ---- main loop: masked row softmax + SpMM, online over sweeps ----
                with (
                    tc.tile_pool(name="adjp", bufs=2) as adjp,
                    tc.tile_pool(name="tp", bufs=2) as tp,
                    tc.tile_pool(name="ttp", bufs=2) as ttp,
                    tc.tile_pool(name="accp", bufs=2) as accp,
                    tc.tile_pool(name="outp", bufs=2) as outp,
                    tc.tile_pool(name="epsp", bufs=2, space="PSUM") as epsp,
                    tc.tile_pool(name="spsp", bufs=2, space="PSUM") as spsp,
                    tc.tile_pool(name="ttpsp", bufs=2, space="PSUM") as ttpsp,
                ):
                    for b in ([] if dbg_skip_main else range(rb)):
                        oacc = accp.tile([128, d], F32, tag="oacc")
                        zacc = vecp.tile([128, 1], F32, tag="zacc")
                        rmrun = None
                        for q in range(nsweep):
                            adj_t = adjp.tile([128, jsweep], F16, tag="adj")
                            nc.sync.dma_start(
                                adj_t[:],
                                adj_d[b * 128:(b + 1) * 128,
                                      q * jsweep:(q + 1) * jsweep])
                            # PSUM seeded with 1000*adj (identity matmul),
                            # e accumulates on top: kept cols sit ~1000 above
                            # masked ones, so exp(psum - rowmax) masks exactly
                            # whole sweep in one 2-bank psum tile: one
                            # negate-fused reduce and one exp for the sweep
                            eps = epsp.tile([128, jsweep], F32, tag="eps")
                            for j in range(jt):
                                joff = q * jsweep + j * 512
                                sl = slice(j * 512, (j + 1) * 512)
                                nc.tensor.matmul(
                                    eps[:, sl], idk_sb[:],
                                    adj_t[:, sl], start=True, stop=False)
                                for c in range(kc):
                                    nc.tensor.matmul(
                                        eps[:, sl],
                                        hT_sb[:, c, b * 128:(b + 1) * 128],
                                        hT_sb[:, c, joff:joff + 512],
                                        start=False, stop=(c == kc - 1))
                            nrmq = vecp.tile([128, 1], F32, tag="nrmq")
                            nc.vector.tensor_reduce(nrmq[:], eps[:], axis=AX.X,
                                                    op=ALU.max, negate=True)
                            t_t = tp.tile([128, jsweep], F16, tag="t")
                            zq = vecp.tile([128, 1], F32, tag="zq")
                            nc.scalar.activation(t_t[:], eps[:], ACTF.Exp,
                                                 bias=nrmq[:], accum_out=zq[:])
                            # transpose t 128-chunks, SpMM against support
                            S = spsp.tile([128, d], F32, tag="S")
                            for g in range(jc // gw):
                                ttps = ttpsp.tile([128, 128 * gw], F16, tag="ttps")
                                for u in range(gw):
                                    ch = g * gw + u
                                    nc.tensor.transpose(
                                        ttps[:, u * 128:(u + 1) * 128],
                                        t_t[:, ch * 128:(ch + 1) * 128], id_sb[:])
                                tt_sb = ttp.tile([128, 128 * gw], F16, tag="tt")
                                nc.vector.tensor_copy(out=tt_sb[:], in_=ttps[:])
                                for u in range(gw):
                                    jchunk = q * jc + g * gw + u
                                    nc.tensor.matmul(
                                        S[:], tt_sb[:, u * 128:(u + 1) * 128],
                                        sup_sb[:, jchunk, :],
                                        start=(g == 0 and u == 0),
                                        stop=(g == jc // gw - 1 and u == gw - 1))
                            if q == 0:
                                nc.vector.tensor_copy(out=oacc[:], in_=S[:])
                                nc.vector.tensor_copy(out=zacc[:], in_=zq[:])
                                rmrun = nrmq
                            else:
                                rmnew = vecp.tile([128, 1], F32, tag="rmnew")
                                nc.vector.tensor_tensor(rmnew[:], rmrun[:], nrmq[:],
                                                        ALU.min)
                                dold = vecp.tile([128, 1], F32, tag="dold")
                                nc.vector.tensor_tensor(dold[:], rmnew[:], rmrun[:],
                                                        ALU.subtract)
                                dq = vecp.tile([128, 1], F32, tag="dq")
                                nc.vector.tensor_tensor(dq[:], rmnew[:], nrmq[:],
                                                        ALU.subtract)
                                cold = vecp.tile([128, 1], F32, tag="cold")
                                nc.scalar.activation(cold[:], dold[:], ACTF.Exp)
                                cq = vecp.tile([128, 1], F32, tag="cq")
                                nc.scalar.activation(cq[:], dq[:], ACTF.Exp)
                                nc.vector.tensor_scalar_mul(oacc[:], oacc[:], cold[:])
                                nc.vector.scalar_tensor_tensor(
                                    out=oacc[:], in0=S[:], scalar=cq[:],
                                    in1=oacc[:], op0=ALU.mult, op1=ALU.add)
                                nc.vector.tensor_scalar_mul(zacc[:], zacc[:], cold[:])
                                nc.vector.scalar_tensor_tensor(
                                    out=zacc[:], in0=zq[:], scalar=cq[:],
                                    in1=zacc[:], op0=ALU.mult, op1=ALU.add)
                                rmrun = rmnew
                        zi = vecp.tile([128, 1], F32, tag="zi")
                        nc.vector.reciprocal(zi[:], zacc[:])
                        scb = vecp.tile([128, 1], F32, tag="scb")
                        nc.vector.tensor_tensor(scb[:], zi[:], scale0[:, b:b + 1],
                                                ALU.mult)
                        o_t = outp.tile([128, d], F32, tag="o")
                        nc.vector.scalar_tensor_tensor(
                            out=o_t[:], in0=oacc[:], scalar=scb[:],
                            in1=bias_sb[:], op0=ALU.mult, op1=ALU.add)
                        nc.sync.dma_start(out_d[b * 128:(b + 1) * 128, :], o_t[:])

    nc.finalize()
    return nc


def make_in_maps(x, adj, weight, bias, node_w, edge_w, n=N, d=D, ncores=NCORES):
    loc = n // ncores
    kc = d // 128
    xt = np.ascontiguousarray(x.T.astype(np.float16)).reshape(kc, 128, n)
    ew = np.ascontiguousarray(edge_w.astype(np.float16)).reshape(kc, 128, d)
    wt = np.ascontiguousarray(weight.astype(np.float16)).reshape(kc, 128, d)
    nw = np.ascontiguousarray(node_w.astype(np.float16)).reshape(kc, 128, 1)
    biasb = np.ascontiguousarray(
        np.broadcast_to(bias.astype(np.float32)[None, :], (128, d)))
    ident = np.eye(128, dtype=np.float16)
    identk = (np.eye(128) * 1000.0).astype(np.float16)
    adj16 = adj.astype(np.float16)
    in_maps = []
    for c in range(ncores):
        sh = c * loc
        xt_c = np.ascontiguousarray(np.roll(xt, -sh, axis=2))
        adj_c = np.ascontiguousarray(np.roll(adj16[sh:sh + loc], -sh, axis=1))
        in_maps.append({"xt": xt_c, "adj": adj_c, "ew": ew, "wt": wt, "nw": nw,
                        "biasb": biasb, "ident": ident, "identk": identk})
    return in_maps


_CACHE = {}


def kernel(x, adj, weight, bias, node_w, edge_w):
    x = np.asarray(x)
    adj = np.asarray(adj)
    weight = np.asarray(weight)
    bias = np.asarray(bias)
    node_w = np.asarray(node_w)
    edge_w = np.asarray(edge_w)
    assert x.shape == (N, D) and adj.shape == (N, N)
    if "nc" not in _CACHE:
        _CACHE["nc"] = build_program()
    nc = _CACHE["nc"]
    in_maps = make_in_maps(x, adj, weight, bias, node_w, edge_w)
    res = run_bass_kernel_spmd(nc, in_maps, list(range(NCORES)))
    out = np.concatenate([res.results[c]["out"] for c in range(NCORES)], axis=0)
    return np.ascontiguousarray(out.astype(np.float32))



# revision 2
# speedup vs baseline: 1.0961x; 1.0961x over previous
"""Trainium2 Bass kernel for DualGraphConvolution — v3 (no collectives, GEMM refactor).

reference math (N=8192, D=512):
    node_att = softmax(x @ node_w, axis=0)            # [N, 1]
    h        = x @ edge_w                             # [N, D]
    e        = h @ h.T ; masked where adj <= 0        # [N, N]
    edge_att = softmax(e, axis=1)                     # [N, N]
    out      = (adj * node_att * edge_att) @ (x @ weight) + bias

Key identities that remove all replicated work and all cross-core traffic:
    e = (x @ M) @ x.T            with M = edge_w @ edge_w.T  (512x512, symmetric)
    (T @ (x @ W)) = (T @ x) @ W  (T = masked/scaled attention weights)
so each core only computes g = x_loc @ M (its 1024 rows) plus tiny M itself,
and streams the raw input x (two layouts: x^T for e, x-rows for T@x) as the
moving operands of both big matmuls.  No AllGather, no replicated h/support.

Row-shard over 8 cores with baseline-style rotation (each core's inputs are
rolled so its own rows come first); all indexing is static/SPMD.

Per 128-row block b, the 8192 columns are processed in two 4096 halves; per
half, four 1024-col sweeps of e land in PSUM and a single fused DVE
tensor_tensor_reduce applies the adjacency mask (adjm = -1000*(1-adj)),
copies e to SBUF fp32, and reduces the row max.  One exp pass per sweep
(bias = -halfmax) makes f16 t tiles -> PE transpose -> (T@x) accumulates in
one PSUM bank per half.  Halves are combined with exp(max_h - max) scales,
then the [128,512] block is transposed (4 PE transposes) and multiplied by W.

Matmul operands are fp16; accumulation/softmax state is fp32.
"""

import numpy as np

import concourse.bass as bass
import concourse.mybir as mybir
import concourse.tile as tile
from concourse import bacc
from concourse.bass_utils import run_bass_kernel_spmd

F16 = mybir.dt.float16
F32 = mybir.dt.float32
ALU = mybir.AluOpType
ACTF = mybir.ActivationFunctionType
AX = mybir.AxisListType

N = 8192
D = 512
NCORES = 8
JS = 1024            # columns per sweep
HALFS = 2
MASK_NEG = -1000.0
NEG_INIT = -3.0e38


def build_program(n=N, d=D, ncores=NCORES, js=JS, dbg=None):
    loc = n // ncores          # 1024 rows owned by this core
    rb = loc // 128            # 8 row blocks per core
    kc = d // 128              # 4 contraction chunks
    nsweep = n // js           # 8
    sph = nsweep // HALFS      # 4
    jc = js // 128             # 8 128-col chunks per sweep

    nc = bacc.Bacc("TRN2", target_bir_lowering=False, debug=False,
                   num_devices=ncores)

    xt_d = nc.dram_tensor("xt", [kc, 128, n], F16, kind="ExternalInput")
    xr_d = nc.dram_tensor("xr", [n // 128, 128, d], F16, kind="ExternalInput")
    adjm_d = nc.dram_tensor("adjm", [loc, n], F16, kind="ExternalInput")
    ewt_d = nc.dram_tensor("ewt", [kc, 128, d], F16, kind="ExternalInput")
    wt_d = nc.dram_tensor("wt", [kc, 128, d], F16, kind="ExternalInput")
    nw_d = nc.dram_tensor("nw", [kc, 128, 1], F16, kind="ExternalInput")
    bias_d = nc.dram_tensor("biasb", [128, d], F32, kind="ExternalInput")
    id_d = nc.dram_tensor("ident", [128, 128], F16, kind="ExternalInput")
    out_d = nc.dram_tensor("out", [loc, d], F32, kind="ExternalOutput")

    with tile.TileContext(nc) as tc:
        with (
            tc.tile_pool(name="const", bufs=1) as constp,
            tc.tile_pool(name="big", bufs=1) as bigp,
            tc.tile_pool(name="vec", bufs=6) as vecp,
        ):
            ewt_sb = constp.tile([128, kc, d], F16)
            nc.sync.dma_start(ewt_sb[:], ewt_d.rearrange("c p f -> p c f"))
            wt_sb = constp.tile([128, kc, d], F16)
            nc.sync.dma_start(wt_sb[:], wt_d.rearrange("c p f -> p c f"))
            nw_sb = constp.tile([128, kc, 1], F16)
            nc.sync.dma_start(nw_sb[:], nw_d.rearrange("c p f -> p c f"))
            bias_sb = constp.tile([128, d], F32)
            nc.sync.dma_start(bias_sb[:], bias_d[:])
            id_sb = constp.tile([128, 128], F16)
            nc.sync.dma_start(id_sb[:], id_d[:])
            ones_row = constp.tile([1, 128], F32)
            nc.vector.memset(ones_row[:], 1.0)

            # streamed input x, one tile per 1024-row group (rotated order):
            # xt_g[g][p, c, r] = x[g*1024+r, c*128+p];  xr_g[g][p, u, f] =
            # x[(g*8+u)*128+p, f]
            xt_g = [bigp.tile([128, kc, js], F16, name=f"xtg{g}")
                    for g in range(ncores)]
            xr_g = [bigp.tile([128, jc, d], F16, name=f"xrg{g}")
                    for g in range(ncores)]
            xt_view = xt_d.rearrange("c p r -> p c r")
            for g in range(ncores):
                q = nc.scalar if g % 2 == 0 else nc.sync
                q.dma_start(
                    xt_g[g][:], xt_view[:, :, g * js:(g + 1) * js])
            for g in range(ncores):
                nc.scalar.dma_start(
                    xr_g[g][:],
                    xr_d[g * jc:(g + 1) * jc].rearrange("rc p f -> p rc f"))

            gT_loc = bigp.tile([128, kc, loc], F16)  # g[r,dd] at [dd%128, dd//128, r]
            pzp = bigp.tile([1, 16], F32)            # per-rt sums of exp(p)
            nc.vector.memset(pzp[:], 0.0)
            scale0 = bigp.tile([128, rb], F32)

            # ---- phase 0: M = Ew Ew^T, g_loc = x_loc M, node attention ----
            with (
                tc.tile_pool(name="ph0", bufs=2) as ph0p,
                tc.tile_pool(name="ph0ps", bufs=2, space="PSUM") as ph0ps,
                tc.tile_pool(name="ph0ps1", bufs=1, space="PSUM") as ph0ps1,
            ):
                m_sb = ph0p.tile([128, kc, d], F16, tag="m")
                for ab in range(kc):
                    mps = ph0ps.tile([128, d], F32, tag="mps")
                    for fc in range(kc):
                        nc.tensor.matmul(
                            mps[:], ewt_sb[:, fc, ab * 128:(ab + 1) * 128],
                            ewt_sb[:, fc, :], start=(fc == 0),
                            stop=(fc == kc - 1))
                    nc.scalar.copy(m_sb[:, ab, :], mps[:])
                # gT_loc: stationary M chunks, moving local x^T
                for rt in range(loc // 512):
                    rsl = slice((rt % 2) * 512, (rt % 2) * 512 + 512)
                    for dc in range(kc):
                        gps = ph0ps.tile([128, 512], F32, tag="gps")
                        for c2 in range(kc):
                            nc.tensor.matmul(
                                gps[:], m_sb[:, c2, dc * 128:(dc + 1) * 128],
                                xt_g[rt // 2][:, c2, rsl],
                                start=(c2 == 0), stop=(c2 == kc - 1))
                        nc.scalar.copy(
                            gT_loc[:, dc, rt * 512:(rt + 1) * 512], gps[:])
            # exp(p) for local rows, kept for the post-pz scaling
            pel = bigp.tile([1, loc], F32)

            # ---- main loop ----
            with (
                tc.tile_pool(name="adjp", bufs=2) as adjp,
                tc.tile_pool(name="ep", bufs=2) as ep,
                tc.tile_pool(name="tp", bufs=2) as tp,
                tc.tile_pool(name="ttp", bufs=2) as ttp,
                tc.tile_pool(name="otp", bufs=2) as otp,
                tc.tile_pool(name="statp", bufs=2) as statp,
                tc.tile_pool(name="pscrp", bufs=1) as pscrp,
                tc.tile_pool(name="outp", bufs=1) as outp,
                tc.tile_pool(name="epsp", bufs=2, space="PSUM") as epsp,
                tc.tile_pool(name="ttpsp", bufs=2, space="PSUM") as ttpsp,
                tc.tile_pool(name="sp", bufs=1, space="PSUM") as spp,
            ):
                for b in range(rb):
                    if dbg != "ph0":
                        mstk = statp.tile([128, nsweep], F32, tag="mstk")
                        zstk = statp.tile([128, nsweep], F32, tag="zstk")
                    S = [spp.tile([128, d], F32, name=f"Sh{h}", tag=f"S{h}")
                         for h in range(HALFS)]
                    nmh = []
                    for h in range(HALFS):
                        e_sb = ep.tile([128, sph, js], F32, tag="esb")
                        for qq in range(sph):
                            q = h * sph + qq
                            adj_t = adjp.tile([128, js], F16, tag="adj")
                            nc.sync.dma_start(
                                adj_t[:],
                                adjm_d[b * 128:(b + 1) * 128,
                                       q * js:(q + 1) * js])
                            eps = epsp.tile([128, js], F32, tag="eps")
                            for j2 in ([] if dbg == "ph0" else range(js // 512)):
                                sl = slice(j2 * 512, (j2 + 1) * 512)
                                for c in range(kc):
                                    nc.tensor.matmul(
                                        eps[:, sl],
                                        gT_loc[:, c, b * 128:(b + 1) * 128],
                                        xt_g[q][:, c, sl],
                                        start=(c == 0), stop=(c == kc - 1))
                            # e_sb = eps + adjm ; mstk = rowmax(e_sb)
                            if dbg == "ph0":
                                continue
                            nc.vector.scalar_tensor_tensor(
                                out=e_sb[:, qq], in0=eps[:], scalar=1.0,
                                in1=adj_t[:], op0=ALU.mult, op1=ALU.add)
                            nc.vector.tensor_reduce(
                                mstk[:, q:q + 1], e_sb[:, qq], axis=AX.X,
                                op=ALU.max)
                            if b == 0:
                                # node attention p = x @ nw for this x group
                                for rr in range(2):
                                    rt = q * 2 + rr
                                    pps_t = epsp.tile([128, js], F32,
                                                      tag="eps")
                                    pps = pps_t[:]
                                    for c in range(kc):
                                        nc.tensor.matmul(
                                            pps[0:1, 0:512],
                                            nw_sb[:, c, :],
                                            xt_g[q][:, c,
                                                    rr * 512:(rr + 1) * 512],
                                            start=(c == 0), stop=(c == kc - 1))
                                    if q == 0:
                                        pdst = pel[:, rt * 512:(rt + 1) * 512]
                                    else:
                                        pscr = pscrp.tile([1, 512], F32,
                                                          tag="pescr")
                                        pdst = pscr[:]
                                    nc.scalar.activation(
                                        pdst, pps[0:1, 0:512], ACTF.Exp,
                                        accum_out=pzp[:, rt:rt + 1])
                        if b == 0 and h == HALFS - 1 and dbg != "ph0":
                            # pz complete: scale0 = exp(p_loc)/pz via K=1 mms
                            pz = vecp.tile([1, 1], F32, tag="pz")
                            nc.vector.reduce_sum(pz[:], pzp[:], axis=AX.X)
                            pzi = vecp.tile([1, 1], F32, tag="pzi")
                            nc.vector.reciprocal(pzi[:], pz[:])
                            sps_t = epsp.tile([128, js], F32, tag="eps")
                            sps_ = sps_t[:]
                            for i in range(rb):
                                nc.tensor.matmul(
                                    sps_[:, i:i + 1],
                                    pel[:, i * 128:(i + 1) * 128],
                                    pzi[:])
                            nc.vector.tensor_copy(out=scale0[:],
                                                  in_=sps_[:, 0:rb])
                        if dbg != "ph0":
                            nmx = vecp.tile([128, 1], F32, tag=f"nmx{h}")
                            nc.vector.tensor_reduce(
                                nmx[:], mstk[:, h * sph:(h + 1) * sph],
                                axis=AX.X, op=ALU.max, negate=True)
                            nmh.append(nmx)
                        for qq in ([] if dbg in ("ephase", "ph0") else range(sph)):
                            q = h * sph + qq
                            t_t = tp.tile([128, js], F16, tag="t")
                            nc.scalar.activation(
                                t_t[:], e_sb[:, qq], ACTF.Exp, bias=nmx[:],
                                accum_out=zstk[:, q:q + 1])
                            ttps = ttpsp.tile([128, js], F16, tag="ttps")
                            for u in range(jc):
                                nc.tensor.transpose(
                                    ttps[:, u * 128:(u + 1) * 128],
                                    t_t[:, u * 128:(u + 1) * 128], id_sb[:])
                            tt_sb = ttp.tile([128, js], F16, tag="tt")
                            if qq % 2 == 0:
                                nc.vector.tensor_copy(out=tt_sb[:], in_=ttps[:])
                            else:
                                nc.scalar.copy(tt_sb[:], ttps[:])
                            for u in range(jc):
                                nc.tensor.matmul(
                                    S[h][:], tt_sb[:, u * 128:(u + 1) * 128],
                                    xr_g[q][:, u, :],
                                    start=(qq == 0 and u == 0),
                                    stop=(qq == sph - 1 and u == jc - 1))
                    if dbg in ("ephase", "ph0"):
                        o_t = outp.tile([128, d], F32, tag="o")
                        nc.vector.memset(o_t[:], 0.0)
                        nc.sync.dma_start(out_d[b * 128:(b + 1) * 128, :], o_t[:])
                        continue
                    # combine halves -> O' = c0*S0 + c1*S1 (f16)
                    nm = vecp.tile([128, 1], F32, tag="nm")
                    nc.vector.tensor_tensor(nm[:], nmh[0][:], nmh[1][:], ALU.min)
                    ch = []
                    for h in range(HALFS):
                        dfh = vecp.tile([128, 1], F32, tag=f"df{h}")
                        nc.vector.tensor_tensor(dfh[:], nm[:], nmh[h][:],
                                                ALU.subtract)
                        cfh = vecp.tile([128, 1], F32, tag=f"cf{h}")
                        nc.scalar.activation(cfh[:], dfh[:], ACTF.Exp)
                        ch.append(cfh)
                    oc_a = otp.tile([128, d], F16, tag="oca")
                    nc.vector.tensor_scalar_mul(oc_a[:], S[0][:], ch[0][:])
                    oc = otp.tile([128, d], F16, tag="oc")
                    nc.vector.scalar_tensor_tensor(
                        out=oc[:], in0=S[1][:], scalar=ch[1][:], in1=oc_a[:],
                        op0=ALU.mult, op1=ALU.add)
                    # O = (O' @ W) * (scale0 / Z) + bias: transpose O', then W
                    otps = ttpsp.tile([128, js], F16, tag="ttps")
                    for fc in range(kc):
                        nc.tensor.transpose(
                            otps[:, fc * 128:(fc + 1) * 128],
                            oc[:, fc * 128:(fc + 1) * 128], id_sb[:])
                    ot_sb = otp.tile([128, d], F16, tag="ot")
                    nc.scalar.copy(ot_sb[:], otps[:, 0:d])
                    ops_t = spp.tile([128, d], F32, tag="S0")
                    for fc in range(kc):
                        nc.tensor.matmul(
                            ops_t[:], ot_sb[:, fc * 128:(fc + 1) * 128],
                            wt_sb[:, fc, :], start=(fc == 0), stop=(fc == kc - 1))
                    # Z and final scale
                    zh = vecp.tile([128, HALFS], F32, tag="zh")
                    nc.vector.tensor_reduce(
                        zh[:], zstk[:].rearrange("p (h q) -> p h q", h=HALFS),
                        axis=AX.X, op=ALU.add)
                    zc0 = vecp.tile([128, 1], F32, tag="zc0")
                    nc.vector.tensor_tensor(zc0[:], zh[:, 0:1], ch[0][:], ALU.mult)
                    Z = vecp.tile([128, 1], F32, tag="Z")
                    nc.vector.scalar_tensor_tensor(
                        out=Z[:], in0=zh[:, 1:2], scalar=ch[1][:], in1=zc0[:],
                        op0=ALU.mult, op1=ALU.add)
                    zi = vecp.tile([128, 1], F32, tag="zi")
                    nc.vector.reciprocal(zi[:], Z[:])
                    sc = vecp.tile([128, 1], F32, tag="sc")
                    nc.vector.tensor_tensor(sc[:], zi[:], scale0[:, b:b + 1],
                                            ALU.mult)
                    o_t = outp.tile([128, d], F32, tag="o")
                    nc.vector.scalar_tensor_tensor(
                        out=o_t[:], in0=ops_t[:], scalar=sc[:],
                        in1=bias_sb[:], op0=ALU.mult, op1=ALU.add)
                    nc.sync.dma_start(out_d[b * 128:(b + 1) * 128, :], o_t[:])

    nc.finalize()
    return nc


def make_in_maps(x, adj, weight, bias, node_w, edge_w, n=N, d=D, ncores=NCORES):
    loc = n // ncores
    kc = d // 128
    ewt = np.ascontiguousarray(edge_w.T.astype(np.float16)).reshape(kc, 128, d)
    wt = np.ascontiguousarray(weight.astype(np.float16)).reshape(kc, 128, d)
    nw = np.ascontiguousarray(node_w.astype(np.float16)).reshape(kc, 128, 1)
    biasb = np.ascontiguousarray(
        np.broadcast_to(bias.astype(np.float32)[None, :], (128, d)))
    ident = np.eye(128, dtype=np.float16)
    x16 = x.astype(np.float16)
    adj16 = adj.astype(np.float16)
    in_maps = []
    for c in range(ncores):
        sh = c * loc
        x_rot = np.roll(x16, -sh, axis=0)
        xt_c = np.ascontiguousarray(x_rot.T).reshape(kc, 128, n)
        xr_c = np.ascontiguousarray(x_rot).reshape(n // 128, 128, d)
        adjm_c = np.ascontiguousarray(
            (np.roll(adj16[sh:sh + loc], -sh, axis=1) - 1) * 1000.0
        ).astype(np.float16)
        in_maps.append({"xt": xt_c, "xr": xr_c, "adjm": adjm_c, "ewt": ewt,
                        "wt": wt, "nw": nw, "biasb": biasb, "ident": ident})
    return in_maps


_CACHE = {}


def kernel(x, adj, weight, bias, node_w, edge_w):
    x = np.asarray(x)
    adj = np.asarray(adj)
    weight = np.asarray(weight)
    bias = np.asarray(bias)
    node_w = np.asarray(node_w)
    edge_w = np.asarray(edge_w)
    assert x.shape == (N, D) and adj.shape == (N, N)
    if "nc" not in _CACHE:
        _CACHE["nc"] = build_program()
    nc = _CACHE["nc"]
    in_maps = make_in_maps(x, adj, weight, bias, node_w, edge_w)
    res = run_bass_kernel_spmd(nc, in_maps, list(range(NCORES)))
    out = np.concatenate([res.results[c]["out"] for c in range(NCORES)], axis=0)
    return np.ascontiguousarray(out.astype(np.float32))
